# revision 31
# baseline (speedup 1.0000x reference)
"""Variant A: 3-pass PE (colsum + carry-broadcast + scan matmuls), minimal DMA.

Same sharding/host-arrangement as v6/v7 (superblock-major contiguous bf16).
Only 16 DMAs total (8 in + 8 out). All carry machinery stays on the PE:
  - phase 1: per group of 16 blocks, one-hot-column matmuls accumulate
    block column-sums S[16, C] into one PSUM bank;
  - phase 2: 4 small matmuls produce T[16, C] (carry + exclusive prefix)
    and the next carry at partition 0;
  - phase 3: per block, a row-selector matmul broadcasts T_i into PSUM
    (start=True), the UT matmul accumulates the in-block prefix, and
    ScalarE/DVE copy PSUM -> bf16 output tiles.
The PE stream is nearly gap-free; if the HAM clock-gate warms this runs at
2.4 GHz (~52 us PE), at 1.2 GHz it is ~103 us.
"""

import numpy as np
import ml_dtypes
from contextlib import ExitStack

import concourse.bass as bass
import concourse.tile as tile
from concourse import bacc, masks, mybir
from concourse.bass_utils import run_bass_kernel_spmd

N_CORES = 8
B, L, D, N = 4, 8192, 32, 32
C_FULL = D * N
C = C_FULL // 2
P = 128
NBLK = L // P
GBLK = 16
NGRP = NBLK // GBLK
SBB = 8
NSB = NBLK // SBB
SBW = SBB * C

_CACHE = {}


def _build_program():
    f32 = mybir.dt.float32
    bf16 = mybir.dt.bfloat16
    nc = bacc.Bacc(
        trn_type="TRN2", debug=False, num_devices=N_CORES, num_swdge_queues=2
    )
    x = nc.dram_tensor("x", [NSB, P, SBW], bf16, kind="ExternalInput").ap()
    y = nc.dram_tensor("y", [NSB, P, SBW], bf16, kind="ExternalOutput").ap()

    with tile.TileContext(nc) as tc, ExitStack() as ctx:
        const_pool = ctx.enter_context(tc.tile_pool(name="const", bufs=1))
        xin_pool = ctx.enter_context(tc.tile_pool(name="xin", bufs=1))
        yout_pool = ctx.enter_context(tc.tile_pool(name="yout", bufs=6))
        small_pool = ctx.enter_context(tc.tile_pool(name="small", bufs=2))
        yps_pool = ctx.enter_context(tc.tile_pool(name="yps", bufs=4, space="PSUM"))
        sps_pool = ctx.enter_context(tc.tile_pool(name="sps", bufs=1, space="PSUM"))
        tps_pool = ctx.enter_context(tc.tile_pool(name="tps", bufs=1, space="PSUM"))
        prm_pool = ctx.enter_context(tc.tile_pool(name="prm", bufs=1, space="PSUM"))

        ut = const_pool.tile([P, P], bf16, name="ut")
        masks.make_upper_triangular(nc, ut[:], 1.0, diag=True)
        # Z1Z: ones in column GBLK-1; a 16-wide slice puts the ones-column
        # at any position 0..15 (phase-1 one-hot stationaries).
        z1z = const_pool.tile([P, 2 * GBLK - 1], bf16, name="z1z")
        nc.gpsimd.memset(z1z[:], 0.0)
        nc.gpsimd.memset(z1z[:, GBLK - 1 : GBLK], 1.0)
        # RZ row-selector bank: slice [:, i*128:(i+1)*128] is all-ones in
        # row i -> matmul replicates T row i onto all 128 output partitions.
        rz = const_pool.tile([GBLK, GBLK * P], bf16, name="rz")
        nc.gpsimd.memset(rz[:], 1.0)
        nc.gpsimd.affine_select(
            out=rz[:], in_=rz[:], compare_op=mybir.AluOpType.is_ge,
            fill=0.0, base=0, pattern=[[1, GBLK * P]], channel_multiplier=-P,
        )
        nc.gpsimd.affine_select(
            out=rz[:], in_=rz[:], compare_op=mybir.AluOpType.is_ge,
            fill=0.0, base=P - 1, pattern=[[-1, GBLK * P]], channel_multiplier=P,
        )
        tms = const_pool.tile([GBLK, GBLK], bf16, name="tms")
        masks.make_upper_triangular(nc, tms[:], 1.0, diag=False)
        ones_1x16 = const_pool.tile([1, GBLK], bf16, name="ones_1x16")
        nc.gpsimd.memset(ones_1x16[:], 1.0)
        ones_16x1 = const_pool.tile([GBLK, 1], bf16, name="ones_16x1")
        nc.gpsimd.memset(ones_16x1[:], 1.0)
        one_1x1 = const_pool.tile([1, 1], bf16, name="one_1x1")
        nc.gpsimd.memset(one_1x1[:], 1.0)
        ca0 = const_pool.tile([1, C], bf16, name="ca0")
        nc.gpsimd.memset(ca0[:], 0.0)

        prev_ca = ca0
        xts = {}

        # issue every in-DMA upfront: X stays fully SBUF-resident (64 KiB of
        # 208 per partition), so once loaded the PE never waits on input and
        # the HAM clock-gate can hold at 8/8.
        for s in range(NSB):
            xt = xin_pool.tile([P, SBW], bf16, name=f"xt{s}", tag=f"xt{s}", bufs=1)
            (nc.sync if s % 2 == 0 else nc.scalar).dma_start(out=xt[:], in_=x[s])
            xts[s] = xt

        # primer matmuls: dependency-free junk work on const tiles. Sprinkled
        # into spots where the real stream would briefly stall (DMA-paced
        # lead-in, carry-copy waits) they keep the HAM activity monitor from
        # re-throttling the PE clock to 4/8.
        prm = prm_pool.tile([P, C], f32, name="prm")

        def primer(n):
            for _ in range(n):
                nc.tensor.matmul(
                    prm[:], rz[:, 0:P], rz[:, 0:C], start=True, stop=True
                )

        def emit_phase1(g, dma_paced=False):
            sp = sps_pool.tile([GBLK, C], f32, name="sp", tag="sp", bufs=1)
            for i in range(GBLK):
                blk = GBLK * g + i
                s, k = blk // SBB, blk % SBB
                nc.tensor.matmul(
                    sp[:],
                    z1z[:, GBLK - 1 - i : 2 * GBLK - 1 - i],
                    xts[s][:, k * C : (k + 1) * C],
                    start=(i == 0),
                    stop=(i == GBLK - 1),
                )
                if dma_paced and k == SBB - 1:
                    primer(4)
            # drain S to SBUF right away so the single S-PSUM bank frees
            sa = small_pool.tile([GBLK, C], bf16, name="sa", tag="sa", bufs=2)
            nc.vector.tensor_copy(sa[:], sp[:])
            return sa

        def emit_carry_math(g, sa):
            nonlocal prev_ca
            ca = prev_ca
            tp = tps_pool.tile([GBLK, C], f32, name="tp", tag="tp", bufs=1)
            nc.tensor.matmul(tp[:], ones_1x16[:], ca[:], start=True, stop=False)
            nc.tensor.matmul(tp[:], tms[:], sa[:], start=False, stop=True)
            tb = small_pool.tile([GBLK, C], bf16, name="tb", tag="tb", bufs=2)
            nc.vector.tensor_copy(tb[:], tp[:])
            if g < NGRP - 1:
                cp = tps_pool.tile([1, C], f32, name="cp", tag="cp", bufs=1)
                nc.tensor.matmul(cp[:], ones_16x1[:], sa[:], start=True, stop=False)
                nc.tensor.matmul(cp[:], one_1x1[:], ca[:], start=False, stop=True)
                nca = small_pool.tile([1, C], bf16, name="nca", tag="nca", bufs=2)
                nc.vector.tensor_copy(nca[:], cp[:])
                prev_ca = nca
            return tb

        def emit_phase3(g, tb):
            yt = None
            # clusters of 4 blocks: 4 carry matmuls, then 4 UT matmuls
            # (consecutive UT matmuls share the stationary load).
            for c0 in range(0, GBLK, 4):
                yps = []
                for i in range(c0, c0 + 4):
                    blk = GBLK * g + i
                    s, k = blk // SBB, blk % SBB
                    if k == 0:
                        yt = yout_pool.tile(
                            [P, SBW], bf16, name=f"yt{s}", tag="yt", bufs=6
                        )
                    yp = yps_pool.tile([P, C], f32, name="yp", tag="yp", bufs=4)
                    nc.tensor.matmul(
                        yp[:], rz[:, i * P : (i + 1) * P], tb[:],
                        start=True, stop=False,
                    )
                    yps.append((yp, yt))
                for j, i in enumerate(range(c0, c0 + 4)):
                    blk = GBLK * g + i
                    s, k = blk // SBB, blk % SBB
                    yp, yti = yps[j]
                    nc.tensor.matmul(
                        yp[:], ut[:], xts[s][:, k * C : (k + 1) * C],
                        start=False, stop=True,
                    )
                    if blk % 2 == 1:
                        nc.vector.tensor_copy(yti[:, k * C : (k + 1) * C], yp[:])
                    else:
                        nc.scalar.copy(yti[:, k * C : (k + 1) * C], yp[:])
                    if k == SBB - 1:
                        (nc.scalar if s % 2 == 0 else nc.sync).dma_start(
                            out=y[s], in_=yti[:]
                        )

        # schedule: ph_0, ph_1, T_0, p3_0, ph_2, T_1, p3_1, ph_3, T_2, p3_2, T_3, p3_3
        primer(8)
        sas = {}
        tbs = {}
        sas[0] = emit_phase1(0, dma_paced=True)
        sas[1] = emit_phase1(1, dma_paced=True)
        tbs[0] = emit_carry_math(0, sas[0])
        primer(2)
        emit_phase3(0, tbs[0])
        sas[2] = emit_phase1(2)
        tbs[1] = emit_carry_math(1, sas[1])
        primer(2)
        emit_phase3(1, tbs[1])
        sas[3] = emit_phase1(3)
        tbs[2] = emit_carry_math(2, sas[2])
        primer(2)
        emit_phase3(2, tbs[2])
        tbs[3] = emit_carry_math(3, sas[3])
        primer(2)
        emit_phase3(3, tbs[3])

    nc.compile()
    return nc


def _get_program():
    if "nc" not in _CACHE:
        _CACHE["nc"] = _build_program()
    return _CACHE["nc"]


def _shard(X):
    Xv = X.reshape(B, L, C_FULL)
    shards = []
    for i in range(N_CORES):
        b, h = i // 2, i % 2
        slab = Xv[b, :, h * C : (h + 1) * C]
        arr = (
            slab.reshape(NSB, SBB, P, C).transpose(0, 2, 1, 3).reshape(NSB, P, SBW)
        )
        shards.append(np.ascontiguousarray(arr).astype(ml_dtypes.bfloat16))
    return shards


def _unshard(parts):
    out = np.empty((B, L, C_FULL), dtype=np.float32)
    for i in range(N_CORES):
        b, h = i // 2, i % 2
        arr = np.asarray(parts[i]).astype(np.float32)
        slab = arr.reshape(NSB, P, SBB, C).transpose(0, 2, 1, 3).reshape(L, C)
        out[b, :, h * C : (h + 1) * C] = slab
    return out.reshape(B, L, D, N)


def kernel(X_in, _trace=False, _tmpdir=None, _trace_cores=None):
    X = np.asarray(X_in, dtype=np.float32)
    assert X.shape == (B, L, D, N), X.shape
    nc = _get_program()
    in_maps = [{"x": s} for s in _shard(X)]
    kwargs = {}
    if _trace:
        kwargs = dict(
            trace=True,
            tmpdir=_tmpdir,
            trace_cores=_trace_cores or list(range(N_CORES)),
        )
    res = run_bass_kernel_spmd(nc, in_maps, core_ids=list(range(N_CORES)), **kwargs)
    out = _unshard([res.results[i]["y"] for i in range(N_CORES)])
    kernel.last_results = res
    return out


# revision 33
# speedup vs baseline: 1.1065x; 1.1065x over previous
"""Variant A: 3-pass PE (colsum + carry-broadcast + scan matmuls), minimal DMA.

Same sharding/host-arrangement as v6/v7 (superblock-major contiguous bf16).
Only 16 DMAs total (8 in + 8 out). All carry machinery stays on the PE:
  - phase 1: per group of 16 blocks, one-hot-column matmuls accumulate
    block column-sums S[16, C] into one PSUM bank;
  - phase 2: 4 small matmuls produce T[16, C] (carry + exclusive prefix)
    and the next carry at partition 0;
  - phase 3: per block, a row-selector matmul broadcasts T_i into PSUM
    (start=True), the UT matmul accumulates the in-block prefix, and
    ScalarE/DVE copy PSUM -> bf16 output tiles.
The PE stream is nearly gap-free; if the HAM clock-gate warms this runs at
2.4 GHz (~52 us PE), at 1.2 GHz it is ~103 us.
"""

import numpy as np
import ml_dtypes
from contextlib import ExitStack

import concourse.bass as bass
import concourse.tile as tile
from concourse import bacc, masks, mybir
from concourse.bass_utils import run_bass_kernel_spmd

N_CORES = 8
B, L, D, N = 4, 8192, 32, 32
C_FULL = D * N
C = C_FULL // 2
P = 128
NBLK = L // P
GBLK = 32
NGRP = NBLK // GBLK
SBB = 8
NSB = NBLK // SBB
SBW = SBB * C

_CACHE = {}


def _build_program():
    f32 = mybir.dt.float32
    bf16 = mybir.dt.bfloat16
    nc = bacc.Bacc(
        trn_type="TRN2", debug=False, num_devices=N_CORES, num_swdge_queues=2
    )
    x = nc.dram_tensor("x", [NSB, P, SBW], bf16, kind="ExternalInput").ap()
    y = nc.dram_tensor("y", [NSB, P, SBW], bf16, kind="ExternalOutput").ap()

    with tile.TileContext(nc) as tc, ExitStack() as ctx:
        const_pool = ctx.enter_context(tc.tile_pool(name="const", bufs=1))
        xin_pool = ctx.enter_context(tc.tile_pool(name="xin", bufs=1))
        yout_pool = ctx.enter_context(tc.tile_pool(name="yout", bufs=6))
        small_pool = ctx.enter_context(tc.tile_pool(name="small", bufs=2))
        yps_pool = ctx.enter_context(tc.tile_pool(name="yps", bufs=5, space="PSUM"))
        sps_pool = ctx.enter_context(tc.tile_pool(name="sps", bufs=1, space="PSUM"))
        tps_pool = ctx.enter_context(tc.tile_pool(name="tps", bufs=1, space="PSUM"))

        ut = const_pool.tile([P, P], bf16, name="ut")
        masks.make_upper_triangular(nc, ut[:], 1.0, diag=True)
        # Z1Z: ones in column GBLK-1; a 16-wide slice puts the ones-column
        # at any position 0..15 (phase-1 one-hot stationaries).
        z1z = const_pool.tile([P, 2 * GBLK - 1], bf16, name="z1z")
        nc.gpsimd.memset(z1z[:], 0.0)
        nc.gpsimd.memset(z1z[:, GBLK - 1 : GBLK], 1.0)
        # RZ row-selector bank: slice [:, i*128:(i+1)*128] is all-ones in
        # row i -> matmul replicates T row i onto all 128 output partitions.
        rz = const_pool.tile([GBLK, GBLK * P], bf16, name="rz")
        nc.gpsimd.memset(rz[:], 1.0)
        nc.gpsimd.affine_select(
            out=rz[:], in_=rz[:], compare_op=mybir.AluOpType.is_ge,
            fill=0.0, base=0, pattern=[[1, GBLK * P]], channel_multiplier=-P,
        )
        nc.gpsimd.affine_select(
            out=rz[:], in_=rz[:], compare_op=mybir.AluOpType.is_ge,
            fill=0.0, base=P - 1, pattern=[[-1, GBLK * P]], channel_multiplier=P,
        )
        tms = const_pool.tile([GBLK, GBLK], bf16, name="tms")
        masks.make_upper_triangular(nc, tms[:], 1.0, diag=False)
        ones_1x16 = const_pool.tile([1, GBLK], bf16, name="ones_1x16")
        nc.gpsimd.memset(ones_1x16[:], 1.0)
        ones_16x1 = const_pool.tile([GBLK, 1], bf16, name="ones_16x1")
        nc.gpsimd.memset(ones_16x1[:], 1.0)
        one_1x1 = const_pool.tile([1, 1], bf16, name="one_1x1")
        nc.gpsimd.memset(one_1x1[:], 1.0)
        ca0 = const_pool.tile([1, C], bf16, name="ca0")
        nc.gpsimd.memset(ca0[:], 0.0)

        prev_ca = ca0
        xts = {}

        # issue every in-DMA upfront: X stays fully SBUF-resident (64 KiB of
        # 208 per partition), so once loaded the PE never waits on input and
        # the HAM clock-gate can hold at 8/8.
        for s in range(NSB):
            xt = xin_pool.tile([P, SBW], bf16, name=f"xt{s}", tag=f"xt{s}", bufs=1)
            (nc.sync if s % 2 == 0 else nc.scalar).dma_start(out=xt[:], in_=x[s])
            xts[s] = xt

        def emit_phase1(g):
            sp = sps_pool.tile([GBLK, C], f32, name="sp", tag="sp", bufs=1)
            for i in range(GBLK):
                blk = GBLK * g + i
                s, k = blk // SBB, blk % SBB
                nc.tensor.matmul(
                    sp[:],
                    z1z[:, GBLK - 1 - i : 2 * GBLK - 1 - i],
                    xts[s][:, k * C : (k + 1) * C],
                    start=(i == 0),
                    stop=(i == GBLK - 1),
                )
            # drain S to SBUF right away so the single S-PSUM bank frees
            sa = small_pool.tile([GBLK, C], bf16, name="sa", tag="sa", bufs=2)
            nc.vector.tensor_copy(sa[:], sp[:])
            return sa

        def emit_carry_math(g, sa):
            nonlocal prev_ca
            ca = prev_ca
            tp = tps_pool.tile([GBLK, C], f32, name="tp", tag="tp", bufs=1)
            nc.tensor.matmul(tp[:], ones_1x16[:], ca[:], start=True, stop=False)
            nc.tensor.matmul(tp[:], tms[:], sa[:], start=False, stop=True)
            tb = small_pool.tile([GBLK, C], bf16, name="tb", tag="tb", bufs=2)
            nc.vector.tensor_copy(tb[:], tp[:])
            if g < NGRP - 1:
                cp = tps_pool.tile([1, C], f32, name="cp", tag="cp", bufs=1)
                nc.tensor.matmul(cp[:], ones_16x1[:], sa[:], start=True, stop=False)
                nc.tensor.matmul(cp[:], one_1x1[:], ca[:], start=False, stop=True)
                nca = small_pool.tile([1, C], bf16, name="nca", tag="nca", bufs=2)
                nc.vector.tensor_copy(nca[:], cp[:])
                prev_ca = nca
            return tb

        def emit_phase3(g, tb):
            yt = None
            for i in range(GBLK):
                blk = GBLK * g + i
                s, k = blk // SBB, blk % SBB
                if k == 0:
                    yt = yout_pool.tile([P, SBW], bf16, name=f"yt{s}", tag="yt", bufs=6)
                yp = yps_pool.tile([P, C], f32, name="yp", tag="yp", bufs=5)
                nc.tensor.matmul(
                    yp[:], rz[:, i * P : (i + 1) * P], tb[:], start=True, stop=False
                )
                nc.tensor.matmul(
                    yp[:], ut[:], xts[s][:, k * C : (k + 1) * C],
                    start=False, stop=True,
                )
                if blk % 2 == 1:
                    nc.vector.tensor_copy(yt[:, k * C : (k + 1) * C], yp[:])
                else:
                    nc.scalar.copy(yt[:, k * C : (k + 1) * C], yp[:])
                if k == SBB - 1:
                    (nc.scalar if s % 2 == 0 else nc.sync).dma_start(
                        out=y[s], in_=yt[:]
                    )

        # schedule: ph_0, ph_1, T_0, p3_0, T_1, p3_1
        sas = {}
        tbs = {}
        sas[0] = emit_phase1(0)
        sas[1] = emit_phase1(1)
        tbs[0] = emit_carry_math(0, sas[0])
        emit_phase3(0, tbs[0])
        tbs[1] = emit_carry_math(1, sas[1])
        emit_phase3(1, tbs[1])

    nc.compile()
    return nc


def _get_program():
    if "nc" not in _CACHE:
        _CACHE["nc"] = _build_program()
    return _CACHE["nc"]


def _shard(X):
    Xv = X.reshape(B, L, C_FULL)
    shards = []
    for i in range(N_CORES):
        b, h = i // 2, i % 2
        slab = Xv[b, :, h * C : (h + 1) * C]
        arr = (
            slab.reshape(NSB, SBB, P, C).transpose(0, 2, 1, 3).reshape(NSB, P, SBW)
        )
        shards.append(np.ascontiguousarray(arr).astype(ml_dtypes.bfloat16))
    return shards


def _unshard(parts):
    out = np.empty((B, L, C_FULL), dtype=np.float32)
    for i in range(N_CORES):
        b, h = i // 2, i % 2
        arr = np.asarray(parts[i]).astype(np.float32)
        slab = arr.reshape(NSB, P, SBB, C).transpose(0, 2, 1, 3).reshape(L, C)
        out[b, :, h * C : (h + 1) * C] = slab
    return out.reshape(B, L, D, N)


def kernel(X_in, _trace=False, _tmpdir=None, _trace_cores=None):
    X = np.asarray(X_in, dtype=np.float32)
    assert X.shape == (B, L, D, N), X.shape
    nc = _get_program()
    in_maps = [{"x": s} for s in _shard(X)]
    kwargs = {}
    if _trace:
        kwargs = dict(
            trace=True,
            tmpdir=_tmpdir,
            trace_cores=_trace_cores or list(range(N_CORES)),
        )
    res = run_bass_kernel_spmd(nc, in_maps, core_ids=list(range(N_CORES)), **kwargs)
    out = _unshard([res.results[i]["y"] for i in range(N_CORES)])
    kernel.last_results = res
    return out
